# revision 1
# baseline (speedup 1.0000x reference)
"""Trainium2 Bass kernel for nn_CrossAttention (B=4, Q=512, KV=2048, H=16 heads,
HID=1024, dh=64), sharded over 8 NeuronCores: data-parallel over batch (4) x
tensor-parallel over heads (2 groups of 8 heads).

Core c = 2*b + g handles batch b, head-group g (hidden slice g*512..g*512+512).

Per-core program (all matmuls in fp32r = full-rate fp32, PSUM fp32):
  - transpose W slices / query / key_value on TensorE (fp32), cast to fp32r in
    the PSUM->SBUF copy
  - qT = (Wq_g @ query.T), kT = (Wk_g @ kv.T)     [hd on partitions]
  - v  = kv @ Wv_g.T with a ones column appended   [kv on partitions]
  - per head: scoresT = kT_h.T @ qT_h (K=64), probsT = exp(scale*s + maskbias)
    on ScalarE straight out of PSUM; attn_outT(+sums row) = vaug.T @ probsT;
    normalize via reciprocal + ones-outer-product broadcast
  - out_part = attn_outT.T @ WoT_g (+ bias on g==0 cores)
  - pairwise ReduceScatter(add) over q rows -> each core returns 256 q rows

kernel(**inputs) takes full inputs, shards on host, runs SPMD on cores 0-7,
and reassembles the (4, 512, 1024) output.
"""

import numpy as np

import concourse.bass as bass
import concourse.mybir as mybir
import concourse.tile as tile
from concourse import bacc
from concourse.bass_utils import run_bass_kernel_spmd
from concourse.masks import make_identity

N_CORES = 8
P = 128
B, Q, KV, HID = 4, 512, 2048, 1024
HDS = 512          # head-dim slice per core (8 heads x 64)
NHEADS = 8         # heads per core
DH = 64
SCALE = 0.125      # 1/sqrt(64)
MASK_BIG = 1e30

F32 = mybir.dt.float32
F32R = mybir.dt.float32r


def _build(loop_k: int = 0, use_f32r: bool = True, analysis: bool = False):
    """Build the SPMD program. loop_k>0 wraps the compute in a For_i hardware
    loop (for timing); the collective + final DMAs stay outside the loop.
    analysis=True builds a 1-core, collective-free variant for TimelineSim."""
    OP_DT = F32R if use_f32r else F32

    nc = bacc.Bacc("TRN2", target_bir_lowering=False, debug=False,
                   num_devices=1 if analysis else N_CORES)

    q_in = nc.dram_tensor("q_in", [Q, HID], F32, kind="ExternalInput")
    kv_in = nc.dram_tensor("kv_in", [KV, HID], F32, kind="ExternalInput")
    wq_in = nc.dram_tensor("wq", [HDS, HID], F32, kind="ExternalInput")
    wk_in = nc.dram_tensor("wk", [HDS, HID], F32, kind="ExternalInput")
    wv_in = nc.dram_tensor("wv", [HDS, HID], F32, kind="ExternalInput")
    wo_in = nc.dram_tensor("wo", [HID, HDS], F32, kind="ExternalInput")
    bo_in = nc.dram_tensor("bo", [1, HID], F32, kind="ExternalInput")
    mask_in = nc.dram_tensor("mask_f", [KV], F32, kind="ExternalInput")
    out_ext = nc.dram_tensor("out", [Q // 2, HID], F32, kind="ExternalOutput")

    cc_in = nc.dram_tensor("cc_in", [Q, HID], F32)
    cc_out = nc.dram_tensor("cc_out", [Q // 2, HID], F32)

    with tile.TileContext(nc) as tc:
        with (
            tc.tile_pool(name="persist", bufs=1) as pp,
            tc.tile_pool(name="weights", bufs=1) as wp,
            tc.tile_pool(name="kvt", bufs=1) as kvp,
            tc.tile_pool(name="raw", bufs=4) as rawp,
            tc.tile_pool(name="probs", bufs=3) as probp,
            tc.tile_pool(name="small", bufs=1) as smallp,
            tc.tile_pool(name="psum_t", bufs=2, space="PSUM") as pst,
            tc.tile_pool(name="psum_p", bufs=2, space="PSUM") as psp,
            tc.tile_pool(name="psum_s", bufs=2, space="PSUM") as pss,
            tc.tile_pool(name="psum_av", bufs=2, space="PSUM") as psav,
        ):
            # ---- static setup (outside any timing loop) ----
            ident = pp.tile([P, P], F32)
            make_identity(nc, ident[:])
            ones1_f = pp.tile([1, P], F32)
            nc.vector.memset(ones1_f[:], 1.0)
            ones1 = pp.tile([1, P], OP_DT)
            nc.vector.tensor_copy(out=ones1[:], in_=ones1_f[:])
            ones8_f = pp.tile([P, NHEADS], F32)
            nc.vector.memset(ones8_f[:], 1.0)
            ones8 = pp.tile([P, NHEADS], OP_DT)
            nc.vector.tensor_copy(out=ones8[:], in_=ones8_f[:])

            def body():
                # ---- mask bias / bias broadcast ----
                mask_sb = pp.tile([P, KV // P], F32, tag="mask_sb")
                nc.sync.dma_start(
                    mask_sb[:], mask_in.ap().rearrange("(n p) -> p n", p=P)
                )
                bias16 = pp.tile([P, KV // P], F32, tag="bias16")
                # (m - 1) * BIG : 0 where mask true, -BIG where false
                nc.vector.tensor_scalar(
                    bias16[:], mask_sb[:], -1.0, MASK_BIG,
                    mybir.AluOpType.add, mybir.AluOpType.mult,
                )

                bo_raw = pp.tile([1, HID], F32, tag="bo_raw")
                nc.sync.dma_start(bo_raw[:], bo_in[:, :])
                bo_r = pp.tile([1, HID], OP_DT, tag="bo_r")
                nc.vector.tensor_copy(out=bo_r[:], in_=bo_raw[:])
                bias_bc = pp.tile([P, HID], F32, tag="bias_bc")
                for ob in range(2):
                    bps = pst.tile([P, 512], F32, tag="tps")
                    nc.tensor.matmul(
                        bps[:], ones1[:, :P], bo_r[:, ob * 512:(ob + 1) * 512],
                        start=True, stop=True,
                    )
                    nc.scalar.copy(bias_bc[:, ob * 512:(ob + 1) * 512], bps[:])

                def transpose_to(dst_slices, raws, kb_range, rb_range, rb_col):
                    """Generic: dst[kb][:, c0+rb*128] = raws[rb][:, kb*128].T"""
                    for kb in kb_range:
                        pt = pst.tile([P, 512], F32, tag="tps")
                        for j, rb in enumerate(rb_range):
                            nc.tensor.transpose(
                                pt[:, j * P:(j + 1) * P],
                                raws[rb][:, kb * P:(kb + 1) * P],
                                ident[:],
                            )
                        dst, c0 = dst_slices(kb)
                        nc.vector.tensor_copy(
                            out=dst[:, c0:c0 + len(rb_range) * P],
                            in_=pt[:, :len(rb_range) * P],
                        )

                # ---- W transposes: wk, wv (wq later), wo ----
                def load_wT(w_dram, tagset):
                    wT = [wp.tile([P, HDS], OP_DT, tag=f"{tagset}T{kb}",
                                  name=f"{tagset}T{kb}")
                          for kb in range(HID // P)]
                    raws = []
                    for rb in range(HDS // P):
                        r = rawp.tile([P, HID], F32, tag="raw4k")
                        nc.sync.dma_start(r[:], w_dram[rb * P:(rb + 1) * P, :])
                        raws.append(r)
                    transpose_to(lambda kb: (wT[kb], 0), raws,
                                 range(HID // P), range(HDS // P), P)
                    return wT

                wkT = load_wT(wk_in, "wk")
                wvT = load_wT(wv_in, "wv")

                # wo: [HID, HDS] -> woT[cb] = [P, HID] (hd on partitions)
                woT = [pp.tile([P, HID], OP_DT, tag=f"woT{cb}", name=f"woT{cb}")
                       for cb in range(HDS // P)]
                for rg in range(2):
                    wo_raws = []
                    for j in range(4):
                        rb = rg * 4 + j
                        r = rawp.tile([P, HDS], F32, tag="raw_wo")
                        nc.sync.dma_start(r[:], wo_in[rb * P:(rb + 1) * P, :])
                        wo_raws.append(r)
                    for cb in range(HDS // P):
                        pt = pst.tile([P, 512], F32, tag="tps")
                        for j in range(4):
                            nc.tensor.transpose(
                                pt[:, j * P:(j + 1) * P],
                                wo_raws[j][:, cb * P:(cb + 1) * P],
                                ident[:],
                            )
                        nc.vector.tensor_copy(
                            out=woT[cb][:, rg * 512:(rg + 1) * 512], in_=pt[:]
                        )

                # ---- persistent attention operands ----
                kT = [pp.tile([P, KV], OP_DT, tag=f"kT{mb}", name=f"kT{mb}")
                      for mb in range(HDS // P)]
                vA = [pp.tile([P, NHEADS * (DH + 1)], OP_DT, tag=f"v{mb}", name=f"v{mb}")
                      for mb in range(KV // P)]

                # ---- kv processed in quarters of 512 rows ----
                for qtr in range(KV // 512):
                    kv_raws = []
                    for rb in range(4):
                        r = rawp.tile([P, HID], F32, tag="raw4k")
                        nc.sync.dma_start(
                            r[:],
                            kv_in[qtr * 512 + rb * P: qtr * 512 + (rb + 1) * P, :],
                        )
                        kv_raws.append(r)
                    kvT = [kvp.tile([P, 512], OP_DT, tag=f"kvT{kb}", name=f"kvT{kb}")
                           for kb in range(HID // P)]
                    transpose_to(lambda kb: (kvT[kb], 0), kv_raws,
                                 range(HID // P), range(4), P)

                    # v-proj: 4 kv-blocks of this quarter
                    for mb4 in range(4):
                        mb = qtr * 4 + mb4
                        vps = psp.tile([P, HDS], F32, tag="proj_ps")
                        for kb in range(HID // P):
                            nc.tensor.matmul(
                                vps[:],
                                kvT[kb][:, mb4 * P:(mb4 + 1) * P],
                                wvT[kb][:],
                                start=(kb == 0), stop=(kb == HID // P - 1),
                            )
                        # strided copy into [h*65 .. h*65+64] lanes + ones col
                        dst = vA[mb][:].rearrange("p (h d) -> p h d", d=DH + 1)
                        src = vps[:].rearrange("p (h d) -> p h d", d=DH)
                        nc.vector.tensor_copy(out=dst[:, :, 0:DH], in_=src[:])
                        nc.vector.tensor_copy(
                            out=dst[:, :, DH:DH + 1],
                            in_=ones8[:].rearrange("p (h o) -> p h o", o=1),
                        )

                    # k-proj: one 512-wide kv chunk
                    for mbh in range(HDS // P):
                        kps = psp.tile([P, 512], F32, tag="proj_ps")
                        for kb in range(HID // P):
                            nc.tensor.matmul(
                                kps[:],
                                wkT[kb][:, mbh * P:(mbh + 1) * P],
                                kvT[kb][:],
                                start=(kb == 0), stop=(kb == HID // P - 1),
                            )
                        nc.vector.tensor_copy(
                            out=kT[mbh][:, qtr * 512:(qtr + 1) * 512],
                            in_=kps[:])

                # ---- query transpose + q-proj ----
                wqT = load_wT(wq_in, "wk")
                qT_raws = []
                for rb in range(Q // P):
                    r = rawp.tile([P, HID], F32, tag="raw4k")
                    nc.sync.dma_start(r[:], q_in[rb * P:(rb + 1) * P, :])
                    qT_raws.append(r)
                queryT = [kvp.tile([P, Q], OP_DT, tag=f"kvT{kb}", name=f"queryT{kb}")
                          for kb in range(HID // P)]
                transpose_to(lambda kb: (queryT[kb], 0), qT_raws,
                             range(HID // P), range(Q // P), P)

                qT = [pp.tile([P, Q], OP_DT, tag=f"qT{mb}", name=f"qT{mb}")
                      for mb in range(HDS // P)]
                for mb in range(HDS // P):
                    qps = psp.tile([P, 512], F32, tag="proj_ps")
                    for kb in range(HID // P):
                        nc.tensor.matmul(
                            qps[:],
                            wqT[kb][:, mb * P:(mb + 1) * P],
                            queryT[kb][:],
                            start=(kb == 0), stop=(kb == HID // P - 1),
                        )
                    nc.vector.tensor_copy(out=qT[mb][:], in_=qps[:])

                # ---- attention per head ----
                attnT = [pp.tile([P, Q], OP_DT, tag=f"attnT{t}", name=f"attnT{t}")
                         for t in range(HDS // P)]
                for h in range(NHEADS):
                    mb = h // 2
                    off = (h % 2) * DH
                    avps = psav.tile([DH + 1, Q], F32, tag="av_ps")
                    for kvb in range(KV // P):
                        sps = pss.tile([P, Q], F32, tag="s_ps")
                        nc.tensor.matmul(
                            sps[:],
                            kT[mb][off:off + DH, kvb * P:(kvb + 1) * P],
                            qT[mb][off:off + DH, :],
                            start=True, stop=True,
                        )
                        probs = probp.tile([P, Q], OP_DT, tag="probs")
                        nc.scalar.activation(
                            probs[:], sps[:],
                            mybir.ActivationFunctionType.Exp,
                            bias=bias16[:, kvb:kvb + 1], scale=SCALE,
                        )
                        nc.tensor.matmul(
                            avps[:],
                            vA[kvb][:, h * (DH + 1):(h + 1) * (DH + 1)],
                            probs[:],
                            start=(kvb == 0), stop=(kvb == KV // P - 1),
                        )
                    recip_f = smallp.tile([1, Q], F32, tag="recip_f")
                    nc.vector.reciprocal(recip_f[:], avps[DH:DH + 1, :])
                    recip = smallp.tile([1, Q], OP_DT, tag="recip")
                    nc.vector.tensor_copy(out=recip[:], in_=recip_f[:])
                    bct = pst.tile([P, 512], F32, tag="tps")
                    nc.tensor.matmul(
                        bct[0:DH, :], ones1[:, :DH], recip[:],
                        start=True, stop=True,
                    )
                    rbc = smallp.tile([DH, Q], F32, tag="rbc")
                    nc.scalar.copy(rbc[:], bct[0:DH, :])
                    nc.vector.tensor_tensor(
                        attnT[mb][off:off + DH, :],
                        avps[0:DH, :], rbc[:],
                        mybir.AluOpType.mult,
                    )

                # ---- out projection + bias ----
                for qb in range(Q // P):
                    for ob in range(2):
                        ops = psp.tile([P, 512], F32, tag="proj_ps")
                        for hdb in range(HDS // P):
                            nc.tensor.matmul(
                                ops[:],
                                attnT[hdb][:, qb * P:(qb + 1) * P],
                                woT[hdb][:, ob * 512:(ob + 1) * 512],
                                start=(hdb == 0), stop=(hdb == HDS // P - 1),
                            )
                        oc = probp.tile([P, 512], F32, tag="out_chunk")
                        nc.vector.tensor_tensor(
                            oc[:], ops[:], bias_bc[:, ob * 512:(ob + 1) * 512],
                            mybir.AluOpType.add,
                        )
                        nc.sync.dma_start(
                            cc_in[qb * P:(qb + 1) * P,
                                  ob * 512:(ob + 1) * 512],
                            oc[:],
                        )

            if loop_k > 0:
                with tc.For_i(0, loop_k):
                    body()
            else:
                body()

            # ---- pairwise reduce-scatter over q rows ----
            if analysis:
                nc.sync.dma_start(out_ext[:, :], cc_in[: Q // 2, :])
            else:
                nc.gpsimd.collective_compute(
                    "ReduceScatter",
                    mybir.AluOpType.add,
                    replica_groups=[[0, 1], [2, 3], [4, 5], [6, 7]],
                    ins=[cc_in.ap().opt()],
                    outs=[cc_out.ap().opt()],
                )
                nc.sync.dma_start(out_ext[:, :], cc_out[:, :])

    nc.compile()
    return nc


_CACHE = {}


def _get_nc(loop_k: int = 0, use_f32r: bool = True):
    key = (loop_k, use_f32r)
    if key not in _CACHE:
        _CACHE[key] = _build(loop_k, use_f32r)
    return _CACHE[key]


def make_in_maps(query, key_value, mask, Wq, Wk, Wv, Wo, bo):
    query = np.asarray(query, dtype=np.float32)
    key_value = np.asarray(key_value, dtype=np.float32)
    mask_f = np.asarray(mask).astype(np.float32)
    Wq = np.asarray(Wq, dtype=np.float32)
    Wk = np.asarray(Wk, dtype=np.float32)
    Wv = np.asarray(Wv, dtype=np.float32)
    Wo = np.asarray(Wo, dtype=np.float32)
    bo = np.asarray(bo, dtype=np.float32)

    in_maps = []
    for c in range(N_CORES):
        b, g = c // 2, c % 2
        sl = slice(g * HDS, (g + 1) * HDS)
        in_maps.append({
            "q_in": np.ascontiguousarray(query[b]),
            "kv_in": np.ascontiguousarray(key_value[b]),
            "wq": np.ascontiguousarray(Wq[sl, :]),
            "wk": np.ascontiguousarray(Wk[sl, :]),
            "wv": np.ascontiguousarray(Wv[sl, :]),
            "wo": np.ascontiguousarray(Wo[:, sl]),
            "bo": (bo if g == 0 else np.zeros_like(bo)).reshape(1, HID),
            "mask_f": np.ascontiguousarray(mask_f[b]),
        })
    return in_maps


def kernel(query, key_value, mask, Wq, Wk, Wv, Wo, bo):
    nc = _get_nc(0, True)
    in_maps = make_in_maps(query, key_value, mask, Wq, Wk, Wv, Wo, bo)
    res = run_bass_kernel_spmd(nc, in_maps, list(range(N_CORES))).results
    out = np.empty((B, Q, HID), dtype=np.float32)
    for b_i in range(B):
        out[b_i, : Q // 2] = res[2 * b_i]["out"]
        out[b_i, Q // 2:] = res[2 * b_i + 1]["out"]
    return out

